# revision 13
# baseline (speedup 1.0000x reference)
"""AttentionPool Trainium2 Bass kernel.

Reference computation (per batch b):
    h      = tanh(x @ W1 + b1)          # [N, H*F]   (big matmul, bf16 on PE)
    scores = h @ W2 + b2                # [N, H]     (PE dot per head chunk)
    scores = where(mask, scores, -1e9)
    w      = softmax(scores, axis=N)    # per head
    pooled = w.T @ x                    # [H, D]
    y      = concat_h(pooled) @ Wout + bout   # [D]

Sharding: data-parallel over batch B=32 across 8 cores (4 batches/core).
Weights replicated. All matmuls in bf16 (fp32 PSUM accumulation); softmax
bias/scale paths in fp32. End-to-end error vs fp32 reference ~4e-3.

Layout notes (per core):
  - x is host-cast to bf16. The score matmul needs x^T (D on partitions):
    loaded via HWDGE DMA-transpose straight from DRAM. The pooling matmul
    needs x natural (N on partitions): loaded as plain DMA.
  - W1 host-prepped to [D, H*F] then chunked [8, 128, HF] (lhsT tiles).
  - scores kept as [4(h), N] rows per batch; softmax reduces over the free
    dim; exp's accum_out gives the denominator for free. The weight
    transpose (n onto partitions) is done on PE via transpose-mode.
"""

import numpy as np
import ml_dtypes

import concourse.bass as bass
import concourse.mybir as mybir
import concourse.tile as tile
from concourse import bacc
from concourse.bass import ts
from concourse.bass_utils import run_bass_kernel_spmd
from concourse.masks import make_identity

BF16 = mybir.dt.bfloat16
FP32 = mybir.dt.float32
AFT = mybir.ActivationFunctionType

P = 128


class Cfg:
    def __init__(self, BL=4, N=2048, D=1024, H=4, F=512, TB=512):
        self.BL, self.N, self.D, self.H, self.F, self.TB = BL, N, D, H, F, TB
        self.HF = H * F
        self.KD = D // P          # k-chunks of D
        self.MC = self.HF // P    # hf-chunks
        self.NBLK = N // TB       # token blocks per batch
        self.NC = N // P          # n-chunks
        self.KOUT = (H * D) // P  # k-chunks of the output projection
        self.R = BL * H           # score rows per core
        assert self.MC % H == 0
        self.FC = self.MC // H    # f-chunks per head


def build_kernel(nc: bass.Bass, cfg: Cfg):
    c = cfg
    x_d = nc.dram_tensor("x", [c.BL, c.N, c.D], BF16, kind="ExternalInput").ap()
    xt_d = nc.dram_tensor("xt", [c.BL, c.KD, P, c.N], BF16, kind="ExternalInput").ap()
    m_d = nc.dram_tensor("m", [c.BL, c.H, c.N], FP32, kind="ExternalInput").ap()
    w1_d = nc.dram_tensor("w1", [c.KD, P, c.HF], BF16, kind="ExternalInput").ap()
    w2_d = nc.dram_tensor("w2", [c.MC, P, c.H], BF16, kind="ExternalInput").ap()
    b1_d = nc.dram_tensor("b1", [c.HF], FP32, kind="ExternalInput").ap()
    wout_d = nc.dram_tensor("wout", [c.KOUT, P, c.D], BF16, kind="ExternalInput").ap()
    bout_d = nc.dram_tensor("bout", [c.BL, c.D], FP32, kind="ExternalInput").ap()
    y_d = nc.dram_tensor("y", [c.BL, c.D], FP32, kind="ExternalOutput").ap()

    with tile.TileContext(nc) as tc:
        with (
            tc.tile_pool(name="const", bufs=1) as const,
            tc.tile_pool(name="xT", bufs=2) as xT_pool,
            tc.tile_pool(name="h", bufs=4) as h_pool,
            tc.tile_pool(name="xn", bufs=3) as xn_pool,
            tc.tile_pool(name="eT", bufs=2) as eT_pool,
            tc.tile_pool(name="sc", bufs=2) as sc_pool,
            tc.tile_pool(name="small", bufs=8) as small_pool,
            tc.tile_pool(name="ysb", bufs=1) as ysb_pool,
            tc.tile_pool(name="hps", bufs=2, space="PSUM") as hps_pool,
            tc.tile_pool(name="scps", bufs=2, space="PSUM") as scps_pool,
            tc.tile_pool(name="tps", bufs=2, space="PSUM") as tps_pool,
            tc.tile_pool(name="plps", bufs=1, space="PSUM") as plps_pool,
        ):
            # ---- constants / weights ----
            w1_sb = const.tile([P, c.KD, c.HF], BF16)
            nc.sync.dma_start(w1_sb[:], w1_d.rearrange("k p f -> p k f"))
            wout_sb = const.tile([P, c.KOUT, c.D], BF16)
            nc.sync.dma_start(wout_sb[:], wout_d.rearrange("k p f -> p k f"))
            w2_sb = const.tile([P, c.MC, c.H], BF16)
            nc.sync.dma_start(w2_sb[:], w2_d.rearrange("c p h -> p c h"))
            b1_sb = const.tile([P, c.MC], FP32)
            nc.sync.dma_start(b1_sb[:], b1_d.rearrange("(c p) -> p c", p=P))
            mask_sb = []
            for b in range(c.BL):
                mt = const.tile([c.H, c.N], FP32, tag=f"mask{b}")
                nc.sync.dma_start(mt[:], m_d[b])
                mask_sb.append(mt)
            bout_sb = const.tile([c.BL, c.D], FP32)
            nc.sync.dma_start(bout_sb[:], bout_d)
            idH_bf = const.tile([c.H, c.H], BF16)
            make_identity(nc, idH_bf[:])
            RP = c.BL * 32  # pooled rows: batch b at partition b*32 + h
            idR_f32 = const.tile([RP, RP], FP32)
            make_identity(nc, idR_f32[:])

            pooled_sb = const.tile([RP, c.D], FP32)
            nc.gpsimd.memset(pooled_sb[:], 0.0)
            poolT_sb = const.tile([P, c.KD, RP], BF16)

            for b in range(c.BL):
                r0 = b * c.H
                sc_sb = sc_pool.tile([c.H, c.N], FP32, tag="scores")
                # ---- scores: h = tanh(x W1 + b1); s = h . W2 ----
                for blk in range(c.NBLK):
                    xT = xT_pool.tile([P, c.KD, c.TB], BF16)
                    nc.sync.dma_start(
                        xT[:], xt_d[b, :, :, ts(blk, c.TB)].rearrange("k p t -> p k t")
                    )
                    sc_ps = scps_pool.tile([c.H, c.TB], FP32)
                    for mc in range(c.MC):
                        h_ps = hps_pool.tile([P, c.TB], FP32)
                        for dc in range(c.KD):
                            nc.tensor.matmul(
                                h_ps[:],
                                w1_sb[:, dc, ts(mc, P)],
                                xT[:, dc, :],
                                start=(dc == 0),
                                stop=(dc == c.KD - 1),
                            )
                        h_sb = h_pool.tile([P, c.TB], BF16)
                        nc.scalar.activation(
                            h_sb[:], h_ps[:], AFT.Tanh, bias=b1_sb[:, mc : mc + 1]
                        )
                        nc.tensor.matmul(
                            sc_ps[:],
                            w2_sb[:, mc, :],
                            h_sb[:],
                            start=(mc == 0),
                            stop=(mc == c.MC - 1),
                        )
                    # scores + (mask + b2)  -> SBUF
                    nc.vector.tensor_add(
                        sc_sb[:, ts(blk, c.TB)],
                        sc_ps[:],
                        mask_sb[b][:, ts(blk, c.TB)],
                    )
                # ---- softmax over N (rows = heads of this batch) ----
                nmx = small_pool.tile([c.H, 1], FP32, tag="nmx")
                nc.vector.tensor_reduce(
                    nmx[:], sc_sb[:], axis=mybir.AxisListType.X,
                    op=mybir.AluOpType.max, negate=True,
                )
                e_sb = sc_pool.tile([c.H, c.N], BF16, tag="e")
                zs = small_pool.tile([c.H, 1], FP32, tag="zs")
                nc.scalar.activation(
                    e_sb[:], sc_sb[:], AFT.Exp, bias=nmx[:], accum_out=zs[:]
                )
                rz = small_pool.tile([c.H, 1], FP32, tag="rz")
                nc.vector.reciprocal(rz[:], zs[:])
                # ---- e^T (n onto partitions) via PE transpose ----
                eT = eT_pool.tile([P, c.NC, c.H], BF16)
                for cn in range(c.NC):
                    tp = tps_pool.tile([P, c.R], BF16, tag="tps")
                    nc.tensor.transpose(
                        tp[:, : c.H], e_sb[:, ts(cn, P)], idH_bf[:]
                    )
                    nc.vector.tensor_copy(eT[:, cn, :], tp[:, : c.H])
                # ---- pooled = (e^T)^T @ x / Z ----
                pl_ps = plps_pool.tile([c.H, c.D], FP32, tag="plps")
                for cn in range(c.NC):
                    xn = xn_pool.tile([P, c.D], BF16)
                    nc.sync.dma_start(xn[:], x_d[b, ts(cn, P), :])
                    for half in range(c.D // 512):
                        nc.tensor.matmul(
                            pl_ps[:, ts(half, 512)],
                            eT[:, cn, :],
                            xn[:, ts(half, 512)],
                            start=(cn == 0),
                            stop=(cn == c.NC - 1),
                        )
                    if c.D < 512:
                        nc.tensor.matmul(
                            pl_ps[:], eT[:, cn, :], xn[:],
                            start=(cn == 0), stop=(cn == c.NC - 1),
                        )
                nc.vector.tensor_scalar_mul(
                    pooled_sb[b * 32 : b * 32 + c.H, :], pl_ps[:], rz[:]
                )

            # ---- pooled^T and output projection ----
            for dc in range(c.KD):
                tp2 = tps_pool.tile([P, RP], FP32, tag="tps")
                nc.tensor.transpose(tp2[:], pooled_sb[:, ts(dc, P)], idR_f32[:])
                nc.vector.tensor_copy(poolT_sb[:, dc, :], tp2[:])
            fin_ps = plps_pool.tile([c.BL, c.D], FP32, tag="plps")
            for k in range(c.KOUT):
                hd, dc = divmod(k, c.KD)
                lhsT = poolT_sb[:, dc, :].rearrange("p (b j) -> p j b", j=32)[:, hd, :]
                nhalf = max(1, c.D // 512)
                w = c.D // nhalf
                for half in range(nhalf):
                    nc.tensor.matmul(
                        fin_ps[:, ts(half, w)],
                        lhsT,
                        wout_sb[:, k, ts(half, w)],
                        start=(k == 0),
                        stop=(k == c.KOUT - 1),
                    )
            y_sb = ysb_pool.tile([c.BL, c.D], FP32)
            nc.vector.tensor_add(y_sb[:], fin_ps[:], bout_sb[:])
            nc.sync.dma_start(y_d[:], y_sb[:])
    return nc


def make_in_maps(x, valid_mask, W1, b1, W2, b2, Wout, bout, n_cores, cfg):
    """Host-side prep: shard over batch, cast/layout weights."""
    c = cfg
    bf16 = ml_dtypes.bfloat16
    x_bf = np.ascontiguousarray(x.astype(bf16))
    # additive mask with b2 baked in, rows = b*H + h
    madd = np.where(valid_mask, np.float32(0), np.float32(-1e9))  # [B, N]
    w1_l = np.ascontiguousarray(
        W1.transpose(1, 0, 2).reshape(c.KD, P, c.HF).astype(bf16)
    )
    w2f = W2.reshape(c.HF).astype(np.float32)
    w2_l = np.zeros((c.MC, P, c.H), np.float32)
    for mc in range(c.MC):
        w2_l[mc, :, mc // c.FC] = w2f[mc * P : (mc + 1) * P]
    w2_l = np.ascontiguousarray(w2_l.astype(bf16))
    b1_l = np.ascontiguousarray(b1.reshape(c.HF).astype(np.float32))
    wout_l = np.ascontiguousarray(Wout.reshape(c.KOUT, P, c.D).astype(bf16))
    bout_l = np.ascontiguousarray(
        np.broadcast_to(bout.astype(np.float32), (c.BL, c.D))
    )
    xt_all = np.ascontiguousarray(x_bf.transpose(0, 2, 1)).reshape(
        x_bf.shape[0], c.KD, P, c.N
    )
    in_maps = []
    for core in range(n_cores):
        b0 = core * c.BL
        m16 = (
            madd[b0 : b0 + c.BL, None, :]
            + b2.astype(np.float32)[None, :, None]
        )  # [BL, H, N]
        in_maps.append(
            {
                "x": np.ascontiguousarray(x_bf[b0 : b0 + c.BL]),
                "xt": np.ascontiguousarray(xt_all[b0 : b0 + c.BL]),
                "m": np.ascontiguousarray(m16.astype(np.float32)),
                "w1": w1_l,
                "w2": w2_l,
                "b1": b1_l,
                "wout": wout_l,
                "bout": bout_l,
            }
        )
    return in_maps


_cached = {}
last_results = None


def kernel(x, valid_mask, W1, b1, W2, b2, Wout, bout, trace=False):
    global last_results
    B = x.shape[0]
    n_cores = 8
    cfg = Cfg(BL=B // n_cores)
    key = (B, trace)
    if "nc" not in _cached:
        nc = bacc.Bacc("TRN2", target_bir_lowering=False, debug=False)
        build_kernel(nc, cfg)
        nc.compile()
        _cached["nc"] = nc
    in_maps = make_in_maps(x, valid_mask, W1, b1, W2, b2, Wout, bout, n_cores, cfg)
    res = run_bass_kernel_spmd(
        _cached["nc"], in_maps, core_ids=list(range(n_cores)), trace=trace
    )
    last_results = res
    y = np.concatenate([res.results[i]["y"] for i in range(n_cores)], axis=0)
    return y.astype(np.float32)


# revision 22
# speedup vs baseline: 1.1009x; 1.1009x over previous
"""AttentionPool Trainium2 Bass kernel.

Reference computation (per batch b):
    h      = tanh(x @ W1 + b1)          # [N, H*F]   (big matmul, bf16 on PE)
    scores = h @ W2 + b2                # [N, H]     (PE dot per head chunk)
    scores = where(mask, scores, -1e9)
    w      = softmax(scores, axis=N)    # per head
    pooled = w.T @ x                    # [H, D]
    y      = concat_h(pooled) @ Wout + bout   # [D]

Sharding: data-parallel over batch B=32 across 8 cores (4 batches/core).
Weights replicated. All matmuls in bf16 (fp32 PSUM accumulation); softmax
bias/scale paths in fp32. End-to-end error vs fp32 reference ~4e-3.

Layout notes (per core):
  - x is host-cast to bf16. The score matmul needs x^T (D on partitions):
    loaded via HWDGE DMA-transpose straight from DRAM. The pooling matmul
    needs x natural (N on partitions): loaded as plain DMA.
  - W1 host-prepped to [D, H*F] then chunked [8, 128, HF] (lhsT tiles).
  - scores kept as [4(h), N] rows per batch; softmax reduces over the free
    dim; exp's accum_out gives the denominator for free. The weight
    transpose (n onto partitions) is done on PE via transpose-mode.
"""

import numpy as np
import ml_dtypes

import concourse.bass as bass
import concourse.mybir as mybir
import concourse.tile as tile
from concourse import bacc
from concourse.bass import ts
from concourse.bass_utils import run_bass_kernel_spmd
from concourse.masks import make_identity

BF16 = mybir.dt.bfloat16
FP32 = mybir.dt.float32
AFT = mybir.ActivationFunctionType

P = 128


class Cfg:
    def __init__(self, BL=4, N=2048, D=1024, H=4, F=512, TB=512):
        self.BL, self.N, self.D, self.H, self.F, self.TB = BL, N, D, H, F, TB
        self.HF = H * F
        self.KD = D // P          # k-chunks of D
        self.MC = self.HF // P    # hf-chunks
        self.NBLK = N // TB       # token blocks per batch
        self.NC = N // P          # n-chunks
        self.KOUT = (H * D) // P  # k-chunks of the output projection
        self.R = BL * H           # score rows per core
        assert self.MC % H == 0
        self.FC = self.MC // H    # f-chunks per head


def build_kernel(nc: bass.Bass, cfg: Cfg):
    c = cfg
    x_d = nc.dram_tensor("x", [c.BL, c.N, c.D], BF16, kind="ExternalInput").ap()
    xt_d = nc.dram_tensor("xt", [c.BL, c.KD, P, c.N], BF16, kind="ExternalInput").ap()
    m_d = nc.dram_tensor("m", [c.BL, c.H, c.N], BF16, kind="ExternalInput").ap()
    w1_d = nc.dram_tensor("w1", [c.KD, P, c.HF], BF16, kind="ExternalInput").ap()
    w2_d = nc.dram_tensor("w2", [c.MC, P, c.H], BF16, kind="ExternalInput").ap()
    b1_d = nc.dram_tensor("b1", [c.HF], FP32, kind="ExternalInput").ap()
    wout_d = nc.dram_tensor("wout", [c.KOUT, P, c.D], BF16, kind="ExternalInput").ap()
    bout_d = nc.dram_tensor("bout", [c.BL, c.D], FP32, kind="ExternalInput").ap()
    y_d = nc.dram_tensor("y", [c.BL, c.D], FP32, kind="ExternalOutput").ap()

    with tile.TileContext(nc) as tc:
        with (
            tc.tile_pool(name="const", bufs=1) as const,
            tc.tile_pool(name="xT", bufs=2) as xT_pool,
            tc.tile_pool(name="h", bufs=6) as h_pool,
            tc.tile_pool(name="xn", bufs=6) as xn_pool,
            tc.tile_pool(name="eT", bufs=2) as eT_pool,
            tc.tile_pool(name="sc", bufs=2) as sc_pool,
            tc.tile_pool(name="small", bufs=8) as small_pool,
            tc.tile_pool(name="ysb", bufs=1) as ysb_pool,
            tc.tile_pool(name="hps", bufs=2, space="PSUM") as hps_pool,
            tc.tile_pool(name="scps", bufs=1, space="PSUM") as scps_pool,
            tc.tile_pool(name="tps", bufs=3, space="PSUM") as tps_pool,
            tc.tile_pool(name="plps", bufs=1, space="PSUM") as plps_pool,
        ):
            # ---- constants / weights ----
            # W1 as 4 independent column-quarter tiles: the first matmul
            # group only waits for quarter 0 (~1MB), the rest stream in
            # behind the first xT block
            QW = c.HF // 4
            w1q = []
            for q in range(4):
                t = const.tile([P, c.KD, QW], BF16, tag=f"w1q{q}")
                w1q.append(t)
            nc.sync.dma_start(
                w1q[0][:], w1_d[:, :, ts(0, QW)].rearrange("k p f -> p k f")
            )
            w2_sb = const.tile([P, c.MC, c.H], BF16)
            b1_sb = const.tile([P, c.MC], FP32)
            mask_sb = [
                const.tile([c.H, c.N], BF16, tag=f"mask{b}", name=f"mask{b}")
                for b in range(c.BL)
            ]
            bout_sb = const.tile([c.BL, c.D], FP32)
            idH_bf = const.tile([c.H, c.H], BF16)
            make_identity(nc, idH_bf[:])
            RP = c.BL * 32  # pooled rows: batch b at partition b*32 + h
            idR_f32 = const.tile([RP, RP], FP32)
            make_identity(nc, idR_f32[:])

            pooled_sb = const.tile([RP, c.D], FP32)
            nc.gpsimd.memset(pooled_sb[:], 0.0)
            poolT_sb = const.tile([P, c.KD, RP], BF16)
            wout_sb = const.tile([P, c.KOUT, c.D], BF16)

            for b in range(c.BL):
                r0 = b * c.H
                if b == 1:
                    # prefetch the output projection during the long middle
                    nc.sync.dma_start(
                        wout_sb[:], wout_d.rearrange("k p f -> p k f")
                    )
                sc_sb = sc_pool.tile([c.H, c.N], FP32, tag="scores")
                blkmax = small_pool.tile([c.H, c.NBLK], FP32, tag="blkmax")
                # ---- scores: h = tanh(x W1 + b1); s = h . W2 ----
                for blk in range(c.NBLK):
                    xT = xT_pool.tile([P, c.KD, c.TB], BF16)
                    nc.sync.dma_start(
                        xT[:], xt_d[b, :, :, ts(blk, c.TB)].rearrange("k p t -> p k t")
                    )
                    if b == 0 and blk == 0:
                        for q in range(1, 4):
                            nc.sync.dma_start(
                                w1q[q][:],
                                w1_d[:, :, ts(q, QW)].rearrange("k p f -> p k f"),
                            )
                        nc.scalar.dma_start(
                            b1_sb[:], b1_d.rearrange("(c p) -> p c", p=P)
                        )
                        nc.scalar.dma_start(
                            w2_sb[:], w2_d.rearrange("c p h -> p c h")
                        )
                        for bb in range(c.BL):
                            nc.scalar.dma_start(mask_sb[bb][:], m_d[bb])
                        nc.scalar.dma_start(bout_sb[:], bout_d)
                    # score partials land in 4 PE column strips
                    # (tile_position col-tiling -> the 4 dots of a round
                    # run concurrently on HW); strips summed on DVE after
                    sc_ps = scps_pool.tile([P, c.TB], FP32)
                    NR = c.MC // 4
                    for rnd in range(NR):
                        h_tiles = []
                        for j in range(4):
                            mc = rnd * 4 + j
                            h_ps = hps_pool.tile([P, c.TB], FP32, tag="h_ps")
                            for dc in range(c.KD):
                                nc.tensor.matmul(
                                    h_ps[:],
                                    w1q[mc // (c.MC // 4)][:, dc, ts(mc % (c.MC // 4), P)],
                                    xT[:, dc, :],
                                    start=(dc == 0),
                                    stop=(dc == c.KD - 1),
                                )
                            h_sb = h_pool.tile([P, c.TB], BF16, tag="h_sb")
                            nc.scalar.activation(
                                h_sb[:], h_ps[:], AFT.Tanh,
                                bias=b1_sb[:, mc : mc + 1],
                            )
                            h_tiles.append(h_sb)
                        for j in range(4):
                            mc = rnd * 4 + j
                            nc.tensor.matmul(
                                sc_ps[32 * j : 32 * j + c.H, :],
                                w2_sb[:, mc, :],
                                h_tiles[j][:],
                                start=(rnd == 0),
                                stop=(rnd == NR - 1),
                                tile_position=(0, 32 * j),
                            )
                    # combine 4 strips + mask -> SBUF (DVE reads at
                    # most one PSUM operand per op, so chain via SBUF)
                    sctmp = small_pool.tile([c.H, c.TB], FP32, tag="sctmp")
                    nc.vector.tensor_copy(sctmp[:], sc_ps[0 : c.H, :])
                    nc.vector.tensor_add(
                        sctmp[:], sctmp[:], sc_ps[32 : 32 + c.H, :]
                    )
                    nc.vector.tensor_add(
                        sctmp[:], sctmp[:], sc_ps[64 : 64 + c.H, :]
                    )
                    nc.vector.tensor_add(
                        sctmp[:], sctmp[:], sc_ps[96 : 96 + c.H, :]
                    )
                    nc.vector.tensor_add(
                        sc_sb[:, ts(blk, c.TB)],
                        sctmp[:],
                        mask_sb[b][:, ts(blk, c.TB)],
                    )
                    nc.vector.tensor_reduce(
                        blkmax[:, blk : blk + 1], sc_sb[:, ts(blk, c.TB)],
                        axis=mybir.AxisListType.X, op=mybir.AluOpType.max,
                    )
                # ---- softmax over N (rows = heads of this batch) ----
                nmx = small_pool.tile([c.H, 1], FP32, tag="nmx")
                nc.vector.tensor_reduce(
                    nmx[:], blkmax[:], axis=mybir.AxisListType.X,
                    op=mybir.AluOpType.max, negate=True,
                )
                e_sb = sc_pool.tile([c.H, c.N], BF16, tag="e")
                zs = small_pool.tile([c.H, 1], FP32, tag="zs")
                nc.scalar.activation(
                    e_sb[:], sc_sb[:], AFT.Exp, bias=nmx[:], accum_out=zs[:]
                )
                rz = small_pool.tile([c.H, 1], FP32, tag="rz")
                nc.vector.reciprocal(rz[:], zs[:])
                # ---- e^T via PE transpose, fused with pooling ----
                eT = eT_pool.tile([P, c.NC, c.H], BF16)
                pl_ps = plps_pool.tile([c.H, c.D], FP32, tag="plps")

                def emit_trans(cn):
                    tp = tps_pool.tile([P, c.R], BF16, tag="tps")
                    nc.tensor.transpose(
                        tp[:, : c.H], e_sb[:, ts(cn, P)], idH_bf[:]
                    )
                    if cn % 2 == 0:
                        nc.vector.tensor_copy(eT[:, cn, :], tp[:, : c.H])
                    else:
                        nc.scalar.copy(eT[:, cn, :], tp[:, : c.H])

                emit_trans(0)
                for cn in range(c.NC):
                    xn = xn_pool.tile([P, c.D], BF16)
                    nc.sync.dma_start(xn[:], x_d[b, ts(cn, P), :])
                    if cn + 1 < c.NC:
                        emit_trans(cn + 1)
                    for half in range(max(1, c.D // 512)):
                        wd = min(512, c.D)
                        nc.tensor.matmul(
                            pl_ps[:, ts(half, wd)],
                            eT[:, cn, :],
                            xn[:, ts(half, wd)],
                            start=(cn == 0),
                            stop=(cn == c.NC - 1),
                        )
                nc.vector.tensor_scalar_mul(
                    pooled_sb[b * 32 : b * 32 + c.H, :], pl_ps[:], rz[:]
                )

            # ---- pooled^T and output projection (col-tiled over heads) ----
            fin_ps = plps_pool.tile([P, c.D], FP32, tag="plps")
            nhalf = max(1, c.D // 512)
            w = c.D // nhalf
            for dc in range(c.KD):
                tp2 = tps_pool.tile([P, RP], FP32, tag="tps")
                nc.tensor.transpose(tp2[:], pooled_sb[:, ts(dc, P)], idR_f32[:])
                nc.vector.tensor_copy(poolT_sb[:, dc, :], tp2[:])
                for hd in range(c.H):
                    k = hd * c.KD + dc
                    lhsT = poolT_sb[:, dc, :].rearrange(
                        "p (b j) -> p j b", j=32
                    )[:, hd, :]
                    for half in range(nhalf):
                        nc.tensor.matmul(
                            fin_ps[32 * hd : 32 * hd + c.BL, ts(half, w)],
                            lhsT,
                            wout_sb[:, k, ts(half, w)],
                            start=(dc == 0),
                            stop=(dc == c.KD - 1),
                            tile_position=(0, 32 * hd),
                        )
            y_sb = ysb_pool.tile([c.BL, c.D], FP32)
            nc.vector.tensor_copy(y_sb[:], fin_ps[0 : c.BL, :])
            nc.vector.tensor_add(y_sb[:], y_sb[:], fin_ps[32 : 32 + c.BL, :])
            nc.vector.tensor_add(y_sb[:], y_sb[:], fin_ps[64 : 64 + c.BL, :])
            nc.vector.tensor_add(y_sb[:], y_sb[:], fin_ps[96 : 96 + c.BL, :])
            nc.vector.tensor_add(y_sb[:], y_sb[:], bout_sb[:])
            nc.sync.dma_start(y_d[:], y_sb[:])
    return nc


def make_in_maps(x, valid_mask, W1, b1, W2, b2, Wout, bout, n_cores, cfg):
    """Host-side prep: shard over batch, cast/layout weights."""
    c = cfg
    bf16 = ml_dtypes.bfloat16
    x_bf = np.ascontiguousarray(x.astype(bf16))
    # additive mask with b2 baked in, rows = b*H + h
    madd = np.where(valid_mask, np.float32(0), np.float32(-1e9))  # [B, N]
    w1_l = np.ascontiguousarray(
        W1.transpose(1, 0, 2).reshape(c.KD, P, c.HF).astype(bf16)
    )
    w2f = W2.reshape(c.HF).astype(np.float32)
    w2_l = np.zeros((c.MC, P, c.H), np.float32)
    for mc in range(c.MC):
        w2_l[mc, :, mc // c.FC] = w2f[mc * P : (mc + 1) * P]
    w2_l = np.ascontiguousarray(w2_l.astype(bf16))
    b1_l = np.ascontiguousarray(b1.reshape(c.HF).astype(np.float32))
    wout_l = np.ascontiguousarray(Wout.reshape(c.KOUT, P, c.D).astype(bf16))
    bout_l = np.ascontiguousarray(
        np.broadcast_to(bout.astype(np.float32), (c.BL, c.D))
    )
    xt_all = np.ascontiguousarray(x_bf.transpose(0, 2, 1)).reshape(
        x_bf.shape[0], c.KD, P, c.N
    )
    # b2 is a per-row constant under the softmax -> it cancels; drop it.
    madd_bf = np.broadcast_to(
        madd.astype(bf16)[:, None, :], (madd.shape[0], c.H, c.N)
    )
    in_maps = []
    for core in range(n_cores):
        b0 = core * c.BL
        in_maps.append(
            {
                "x": np.ascontiguousarray(x_bf[b0 : b0 + c.BL]),
                "xt": np.ascontiguousarray(xt_all[b0 : b0 + c.BL]),
                "m": np.ascontiguousarray(madd_bf[b0 : b0 + c.BL]),
                "w1": w1_l,
                "w2": w2_l,
                "b1": b1_l,
                "wout": wout_l,
                "bout": bout_l,
            }
        )
    return in_maps


_cached = {}
last_results = None


def kernel(x, valid_mask, W1, b1, W2, b2, Wout, bout, trace=False):
    global last_results
    B = x.shape[0]
    n_cores = 8
    cfg = Cfg(BL=B // n_cores)
    key = (B, trace)
    if "nc" not in _cached:
        nc = bacc.Bacc("TRN2", target_bir_lowering=False, debug=False)
        build_kernel(nc, cfg)
        nc.compile()
        _cached["nc"] = nc
    in_maps = make_in_maps(x, valid_mask, W1, b1, W2, b2, Wout, bout, n_cores, cfg)
    res = run_bass_kernel_spmd(
        _cached["nc"], in_maps, core_ids=list(range(n_cores)), trace=trace
    )
    last_results = res
    y = np.concatenate([res.results[i]["y"] for i in range(n_cores)], axis=0)
    return y.astype(np.float32)
